# revision 1
# baseline (speedup 1.0000x reference)
"""4D SAME cross-correlation (H,W,D,F spatial) on 8 Trainium2 cores.

Formulation: banded matmul over the frame axis.
  out[(fo,co), (h,w,d)] = sum over 27 spatial taps (fh,fw,fd) of
      Wb_tap[(fi,ci), (fo,co)]^T @ x_slab[(fi,ci), (h+fh, w+fw, d+fd)]
where Wb_tap is the frame-banded weight (nonzero iff ff = fi-fo in [0,3))
and a 97th contraction row of ones carries the bias (folded into tap 0).

Sharding: 8 cores = 2 batch x 4 frame-blocks of 4 output frames each.
Each core's input slab is the 6-frame padded window, host-pretransposed to
[(fi,ci)=96 (+1 ones row), padded (h,w,d) = 34^3], bf16. Spatial shifts are
free-dim AP offsets into the padded slab -- no im2col copies on device.
"""

import numpy as np
import ml_dtypes

import concourse.bass as bass
import concourse.mybir as mybir
import concourse.tile as tile
from concourse.bass_utils import run_bass_kernel_spmd

N, H, W, D, F, CIN = 2, 32, 32, 32, 16, 16
COUT = 32
FB = 4                 # output frames per core
FI = FB + 2            # input frame window per core
K = FI * CIN + 1       # 97 (incl. ones/bias row)
M = FB * COUT          # 128
HP, WP, DP = H + 2, W + 2, D + 2
NPAD = HP * WP * DP    # 39304
NPOS = H * W * D       # 32768
NT = 512               # one PSUM bank (fp32)
NCORES = 8
BF16 = mybir.dt.bfloat16

_cache = {}


def _emit():
    nc = bass.Bass()
    xs = nc.declare_dram_parameter("xs", [K, NPAD], BF16, isOutput=False)
    wb = nc.declare_dram_parameter("wb", [K, 27 * M], BF16, isOutput=False)
    out = nc.declare_dram_parameter("out", [M, NPOS], mybir.dt.float32,
                                    isOutput=True)
    with tile.TileContext(nc) as tc:
        with (
            tc.tile_pool(name="xsp", bufs=1) as xsp,
            tc.tile_pool(name="wp", bufs=1) as wpp,
            tc.tile_pool(name="ps", bufs=8, space="PSUM") as psp,
            tc.tile_pool(name="tmp", bufs=2) as tmpp,
            tc.tile_pool(name="ob", bufs=4) as obp,
        ):
            xs_t = xsp.tile([K, NPAD], BF16)
            nch = 8
            csz = NPAD // nch  # 4913
            for i in range(nch):
                nc.gpsimd.dma_start(out=xs_t[:, i * csz:(i + 1) * csz],
                                  in_=xs[:, i * csz:(i + 1) * csz])
            w_t = wpp.tile([K, 27 * M], BF16)
            nc.gpsimd.dma_start(out=w_t[:], in_=wb[:])

            xs_v = xs_t[:].rearrange("p (h w d) -> p h w d", h=HP, w=WP, d=DP)

            # out column order: (h, dhalf, w, dlo) so each N-tile's store is
            # a contiguous [M, 512] DMA (strided DRAM writes overflow the
            # direct2d descriptor's sync-wait table).
            for nt in range(NPOS // NT):
                h0, d0 = nt // 2, (nt % 2) * 16
                ps_t = psp.tile([M, NT], mybir.dt.float32)
                ps_v = ps_t[:].rearrange("m (w d) -> m w d", w=W, d=16)
                for t in range(27):
                    fh, fw, fd = t // 9, (t // 3) % 3, t % 3
                    rhs = xs_v[:, h0 + fh, fw:fw + W, d0 + fd:d0 + fd + 16]
                    nc.tensor.matmul(ps_v, w_t[:, t * M:(t + 1) * M], rhs,
                                     start=(t == 0), stop=(t == 26))
                # two-stage PSUM drain: the verified-on-HW configuration
                # (single-copy variant hit NRT_EXEC_UNIT_UNRECOVERABLE)
                tmp_t = tmpp.tile([M, NT], mybir.dt.float32)
                nc.vector.tensor_copy(tmp_t[:], ps_t[:])
                ob_t = obp.tile([M, NT], mybir.dt.float32)
                nc.vector.tensor_copy(ob_t[:], tmp_t[:])
                nc.sync.dma_start(out=out[:, nt * NT:(nt + 1) * NT],
                                  in_=ob_t[:])
    return nc


def _legalize_waits(nc):
    """walrus codegen fits only one sem-wait slot per TPB instruction; hoist
    extra waits onto standalone EventSemaphore instructions on the same
    engine, placed immediately before the instruction they guard."""
    for bb in nc.m.functions[0].blocks:
        new = []
        for ins in bb.instructions:
            si = ins.sync_info
            if si is not None and len(si.on_wait) > 1:
                for w in si.on_wait[1:]:
                    new.append(mybir.InstEventSemaphore(
                        name=nc.get_next_instruction_name(),
                        engine=ins.engine,
                        ins=[], outs=[],
                        sync_info=mybir.SyncInfo(on_wait=[w], on_update=[]),
                    ))
                ins.sync_info = mybir.SyncInfo(on_wait=[si.on_wait[0]],
                                               on_update=si.on_update)
            new.append(ins)
        bb.instructions = new


def _prep(x, kernel, bias):
    xp = np.pad(x, ((0, 0), (1, 1), (1, 1), (1, 1), (1, 1), (0, 0)))
    slabs = []
    for c in range(NCORES):
        n, k = c // 4, c % 4
        s = xp[n, :, :, :, 4 * k:4 * k + FI, :]          # [34,34,34,6,16]
        s = np.transpose(s, (3, 4, 0, 1, 2)).reshape(FI * CIN, NPAD)
        s = np.concatenate([s, np.ones((1, NPAD), np.float32)], axis=0)
        slabs.append(s.astype(ml_dtypes.bfloat16))
    wbh = np.zeros((K, 27 * M), np.float32)
    for t in range(27):
        fh, fw, fd = t // 9, (t // 3) % 3, t % 3
        for fo in range(FB):
            for ff in range(3):
                fi = fo + ff
                wbh[fi * CIN:(fi + 1) * CIN, t * M + fo * COUT:(t * M + (fo + 1) * COUT)] = \
                    kernel[fh, fw, fd, ff]
    wbh[K - 1, 0 * M:1 * M] = np.tile(bias.reshape(COUT), FB)
    return slabs, wbh.astype(ml_dtypes.bfloat16)


def _run(x, kernel, bias, trace=False):
    if "nc" not in _cache:
        nc = _emit()
        _legalize_waits(nc)
        _cache["nc"] = nc
    nc = _cache["nc"]
    slabs, wbh = _prep(np.asarray(x, np.float32), np.asarray(kernel, np.float32),
                       np.asarray(bias, np.float32))
    in_maps = [{"xs": slabs[c], "wb": wbh} for c in range(NCORES)]
    res = run_bass_kernel_spmd(nc, in_maps, list(range(NCORES)), trace=trace)
    full = np.empty((N, H, W, D, F, COUT), np.float32)
    for c in range(NCORES):
        n, k = c // 4, c % 4
        o = res.results[c]["out"].reshape(FB, COUT, H, 2, W, 16)
        o = np.transpose(o, (2, 4, 3, 5, 0, 1)).reshape(H, W, D, FB, COUT)
        full[n, :, :, :, 4 * k:4 * k + FB, :] = o
    return full, res


def kernel(x, kernel, bias):
    return _run(x, kernel, bias, trace=False)[0]

